# revision 15
# baseline (speedup 1.0000x reference)
"""2-layer relational GCN (RGCN) on Trainium2, 8-core SPMD — v2.

Sharding: edges partitioned by dst-node range (core c owns dst nodes
[c*N/8, (c+1)*N/8)); node features and weights replicated. Self-loops are
computed densely (featT @ loop1 folded into the aggregation PSUM), not as
edges.

All irregular access goes through the batched `dma_gather` custom
instruction (≤2048 rows per instruction) instead of one 128-row indirect
DMA per tile; messages are written back contiguously and re-gathered in
dst order (gathers have no write hazards, unlike scatter-add whose CCE
read-modify-write loses colliding updates). dma_gather indices are int16,
so gathers from the 50000-row feat/h tables are split at row 32768
("halves") and the per-core message buffer is split into NPART dst-range
parts of <=32767 rows.

Per core:
  L1 messages: per (half, rel)-grouped 128-slot tile, gather feat16[src]
    (fp16, 256B rows) -> PE transpose -> matmul W1[r] -> contiguous write
    to msgbuf.
  L1 aggregation (per part): gather msgbuf rows in dst-tile order ->
    one-hot sel matmuls accumulating in PSUM + dense self-loop matmul
    (featT16 @ loop1) -> +b1, ReLU -> h shard (fp16, padded to 128 cols);
    also PE-transpose h tiles into SBUF for the L2 self-loop.
  AllGather h16 shards.
  L2: gather h16[src] in (half, dst-tile) order -> transpose -> matmul all
    19 relations at once (W2 flat, 38 cols) -> etype mask + reduce ->
    one-hot sel matmuls; half-0 partials parked in SBUF, half-1 session
    adds self-loop (hT @ loop2) in PSUM, final = h0 + h1 + b2.
"""

import numpy as np

P = 128
C = 8
HALF = 32768       # int16 index limit for dma_gather
GCH = 8            # tiles per dma_gather chunk (8*128 = 1024 idxs; >1 in-flight
                   # 2048-idx custom gathers overflow the SWDGE ring on HW)
MB = 4             # tiles per PSUM batch / copy batch

PHASES = 3          # 1=L1 only, 2=+aggregation, 3=full
NO_COLLECTIVE = False

_CACHE = {}


# ---------------------------------------------------------------- host prep

def _wrap16(idx):
    """[n] -> [128, n//16] int16 wrap-16 layout replicated to 128 parts."""
    n = len(idx)
    assert n % 16 == 0
    w = np.asarray(idx, np.int16).reshape(n // 16, 16).T
    return np.ascontiguousarray(np.tile(w, (8, 1)))


def _tmaj(a):
    """[n_slots] -> [128, n_tiles] tile-major f32."""
    return np.ascontiguousarray(
        np.asarray(a, np.float32).reshape(-1, P).T)


def _group_slots(order_vals, caps, offs):
    """Given per-edge group ids (sorted stably by group) assign slots:
    group g's edges go to offs[g], offs[g]+1, ...  Returns slot per edge
    (in the *original* edge order of order_vals)."""
    n = len(order_vals)
    o = np.argsort(order_vals, kind="stable")
    g_sorted = order_vals[o]
    starts = np.zeros(len(caps), np.int64)
    cnt = np.bincount(g_sorted, minlength=len(caps))
    starts[1:] = np.cumsum(cnt)[:-1]
    slot_sorted = offs[g_sorted] + (np.arange(n) - starts[g_sorted])
    slot = np.empty(n, np.int64)
    slot[o] = slot_sorted
    return slot


def _preprocess(feat, W1, loop1, b1, W2, loop2, b2, src, dst, etype):
    feat = np.asarray(feat, dtype=np.float32)
    W1 = np.asarray(W1, dtype=np.float32)
    W2 = np.asarray(W2, dtype=np.float32)
    loop1 = np.asarray(loop1, dtype=np.float32)
    loop2 = np.asarray(loop2, dtype=np.float32)
    b1 = np.asarray(b1, dtype=np.float32)
    b2 = np.asarray(b2, dtype=np.float32)
    src = np.asarray(src).astype(np.int64).ravel()
    dst = np.asarray(dst).astype(np.int64).ravel()
    etype = np.asarray(etype).astype(np.int64).ravel()

    N, D = feat.shape
    R, _, H = W1.shape
    O = W2.shape[2]
    assert D == P and N % C == 0
    S = N // C
    NT = -(-S // P)                      # dst tiles per core (49)
    core_of = dst // S
    halves = (src >= HALF).astype(np.int64)

    per_core = []
    for c in range(C):
        m = core_of == c
        per_core.append((src[m], dst[m] - c * S, etype[m], halves[m]))

    # ---- choose NPART so each part's L1 slot count fits int16
    for NPART in (3, 4, 5, 6):
        bounds = np.linspace(0, NT, NPART + 1).astype(np.int64)
        ok = True
        cap1 = np.zeros((NPART, 2, R), np.int64)
        for c in range(C):
            es, ed, ee, eh = per_core[c]
            pt = np.searchsorted(bounds, ed // P, side="right") - 1
            g = (pt * 2 + eh) * R + ee
            cnt = np.bincount(g, minlength=NPART * 2 * R)
            cap1 = np.maximum(cap1, cnt.reshape(NPART, 2, R))
        gtiles1 = -(-cap1 // P)          # [NPART, 2, R] tiles per group
        T1p = gtiles1.reshape(NPART, -1).sum(1)
        if (T1p * P).max() <= 32600:
            break
        ok = False
    assert ok or (T1p * P).max() <= 32600, T1p

    # L1 layout (shared): concat parts; per part, groups ordered (half, rel)
    T1tot = int(T1p.sum())
    tile_rel = np.zeros(T1tot, np.int64)
    l1_chunks = []                       # (tile0, ntiles, halfsel)
    goff1 = np.zeros((NPART, 2, R), np.int64)   # slot offset of group
    moff = np.zeros(NPART, np.int64)     # msgbuf row offset of part
    t = 0
    for p in range(NPART):
        moff[p] = t * P
        for h in (0, 1):
            h0 = t
            for r in range(R):
                goff1[p, h, r] = t * P
                tile_rel[t:t + gtiles1[p, h, r]] = r
                t += gtiles1[p, h, r]
            for u in range(h0, t, GCH):
                l1_chunks.append((u, min(GCH, t - u), h))
    msg_rows = T1tot * P

    # ---- phase-2 (L1 aggregation): per part, dst-tile sessions
    cnt2 = np.zeros((C, NT), np.int64)
    for c in range(C):
        es, ed, ee, eh = per_core[c]
        cnt2[c] += np.bincount(ed // P, minlength=NT)
    k2 = np.maximum(1, -(-cnt2.max(0) // P))     # chunks per dst tile
    p2_tiles = []                        # (gt, k, klast, part)
    p2_chunks = []                       # (tile0, ntiles, part)
    toff2 = np.zeros(NT, np.int64)       # slot offset of dst tile (global)
    t = 0
    for p in range(NPART):
        p0 = t
        for gt in range(int(bounds[p]), int(bounds[p + 1])):
            toff2[gt] = t * P
            for k in range(int(k2[gt])):
                p2_tiles.append((gt, k, k == k2[gt] - 1, p))
                t += 1
        for u in range(p0, t, GCH):
            p2_chunks.append((u, min(GCH, t - u), p))
    T2tot = t

    # ---- L2: (half, dst-tile) sessions
    cnt3 = np.zeros((C, 2, NT), np.int64)
    for c in range(C):
        es, ed, ee, eh = per_core[c]
        np.add.at(cnt3[c], (eh, ed // P), 1)
    k3 = np.maximum(1, -(-cnt3.max(0) // P))     # [2, NT]
    l3_tiles = []                        # (gt, k, klast, half)
    l3_chunks = []                       # (tile0, ntiles, half)
    toff3 = np.zeros((2, NT), np.int64)
    t = 0
    for h in (0, 1):
        h0 = t
        for gt in range(NT):
            toff3[h, gt] = t * P
            for k in range(int(k3[h, gt])):
                l3_tiles.append((gt, k, k == k3[h, gt] - 1, h))
                t += 1
        for u in range(h0, t, GCH):
            l3_chunks.append((u, min(GCH, t - u), h))
    T3tot = t

    # ---- replicated tensors
    feat16 = feat.astype(np.float16)
    w1f16 = np.ascontiguousarray(
        W1.transpose(1, 0, 2).reshape(D, R * H)).astype(np.float16)
    w2f16 = np.ascontiguousarray(
        W2.transpose(1, 0, 2).reshape(H, R * O)).astype(np.float16)
    loop1_16 = loop1.astype(np.float16)                    # [128, 64]
    loop2_16 = loop2.astype(np.float16)                    # [64, 2]
    b1b = np.ascontiguousarray(np.broadcast_to(b1, (P, H))).copy()
    b2b = np.ascontiguousarray(np.broadcast_to(b2, (P, O))).copy()

    in_maps = []
    for c in range(C):
        es, ed, ee, eh = per_core[c]
        nE = len(es)
        pt = np.searchsorted(bounds, ed // P, side="right") - 1
        ftT = np.zeros((P, NT * P), np.float16)
        ftT[:, :S] = feat[c * S:(c + 1) * S].T.astype(np.float16)

        # L1 slots: group (part, half, rel)
        g1 = (pt * 2 + eh) * R + ee
        slot1 = _group_slots(g1, cap1.ravel(), goff1.ravel())
        g1t = np.zeros(T1tot * P, np.int16)
        g1t[slot1] = (es - eh * HALF).astype(np.int16)

        # phase-2 slots: group by dst tile
        slot2 = _group_slots(ed // P, cnt2.max(0), toff2)
        s2g = np.zeros(T2tot * P, np.int16)
        d2 = np.full(T2tot * P, -1.0, np.float32)
        s2g[slot2] = (slot1 - moff[pt]).astype(np.int16)
        d2[slot2] = (ed % P).astype(np.float32)

        # L2 slots: group by (half, dst tile)
        slot3 = _group_slots(eh * NT + ed // P, cnt3.max(0).ravel(),
                             toff3.ravel())
        g3 = np.zeros(T3tot * P, np.int16)
        d3 = np.full(T3tot * P, -1.0, np.float32)
        e3 = np.full(T3tot * P, -1.0, np.float32)
        g3[slot3] = (es - eh * HALF).astype(np.int16)
        d3[slot3] = (ed % P).astype(np.float32)
        e3[slot3] = ee.astype(np.float32)

        in_maps.append({
            "feat16": feat16, "featT16": ftT, "w1f16": w1f16,
            "w2f16": w2f16, "loop1s": loop1_16, "loop2s": loop2_16,
            "b1b": b1b, "b2b": b2b,
            "g1w": _wrap16(g1t), "s2gw": _wrap16(s2g), "g3w": _wrap16(g3),
            "d2t": _tmaj(d2).astype(np.float16),
            "d3t": _tmaj(d3).astype(np.float16), "e3t": _tmaj(e3),
        })

    plan = dict(
        N=N, D=D, H=H, O=O, R=R, S=S, NT=NT, NPART=NPART,
        T1tot=T1tot, T2tot=T2tot, T3tot=T3tot, msg_rows=msg_rows,
        tile_rel=tuple(int(x) for x in tile_rel),
        l1_chunks=tuple(l1_chunks), p2_chunks=tuple(p2_chunks),
        l3_chunks=tuple(l3_chunks),
        p2_tiles=tuple(p2_tiles), l3_tiles=tuple(l3_tiles),
        moff=tuple(int(x) for x in moff),
        T1p=tuple(int(x) for x in T1p),
    )
    return plan, in_maps


# ---------------------------------------------------------------- device prog

def _bc_inner(ap, n):
    """[P, c] -> [P, c, n], broadcasting the new innermost dim."""
    import concourse.bass as bass
    return bass.AP(ap.tensor, ap.offset, list(ap.ap) + [[0, n]])


def _shrink_last(ap, n):
    """[..., m] -> [..., n] view (n <= m), keeping strides."""
    import concourse.bass as bass
    a = list(ap.ap)
    a[-1] = [a[-1][0], n]
    return bass.AP(ap.tensor, ap.offset, a)


def _bc_mid(ap, g):
    """[P, f] -> [P, g, f], broadcasting the new middle dim."""
    import concourse.bass as bass
    a = list(ap.ap)
    return bass.AP(ap.tensor, ap.offset, [a[0], [0, g]] + a[1:])


def _build(plan):
    import concourse.bacc as bacc
    import concourse.tile as tile
    import concourse.mybir as mybir
    from concourse.masks import make_identity

    N, D, H, O, R = plan["N"], plan["D"], plan["H"], plan["O"], plan["R"]
    S, NT, NPART = plan["S"], plan["NT"], plan["NPART"]
    T1tot, T2tot, T3tot = plan["T1tot"], plan["T2tot"], plan["T3tot"]
    msg_rows = plan["msg_rows"]
    tile_rel = plan["tile_rel"]
    l1_chunks, p2_chunks, l3_chunks = (plan["l1_chunks"], plan["p2_chunks"],
                                       plan["l3_chunks"])
    p2_tiles, l3_tiles = plan["p2_tiles"], plan["l3_tiles"]
    moff, T1p = plan["moff"], plan["T1p"]
    f32 = mybir.dt.float32
    f16 = mybir.dt.float16
    i16 = mybir.dt.int16
    i32 = mybir.dt.int32
    AO = mybir.AluOpType
    RO = R * O

    nc = bacc.Bacc("TRN2", target_bir_lowering=False, debug=False,
                   num_devices=C, num_swdge_queues=4)
    feat16 = nc.dram_tensor("feat16", [N, D], f16, kind="ExternalInput")
    featT16 = nc.dram_tensor("featT16", [P, NT * P], f16,
                             kind="ExternalInput")
    w1f16 = nc.dram_tensor("w1f16", [D, R * H], f16, kind="ExternalInput")
    w2f16 = nc.dram_tensor("w2f16", [H, RO], f16, kind="ExternalInput")
    loop1d = nc.dram_tensor("loop1s", [D, H], f16, kind="ExternalInput")
    loop2d = nc.dram_tensor("loop2s", [H, O], f16, kind="ExternalInput")
    b1d = nc.dram_tensor("b1b", [P, H], f32, kind="ExternalInput")
    b2d = nc.dram_tensor("b2b", [P, O], f32, kind="ExternalInput")
    g1d = nc.dram_tensor("g1w", [P, T1tot * P // 16], i16,
                         kind="ExternalInput")
    s2gd = nc.dram_tensor("s2gw", [P, T2tot * P // 16], i16,
                          kind="ExternalInput")
    g3d = nc.dram_tensor("g3w", [P, T3tot * P // 16], i16,
                         kind="ExternalInput")
    d2d = nc.dram_tensor("d2t", [P, T2tot], f16, kind="ExternalInput")
    d3d = nc.dram_tensor("d3t", [P, T3tot], f16, kind="ExternalInput")
    e3d = nc.dram_tensor("e3t", [P, T3tot], f32, kind="ExternalInput")
    outs = nc.dram_tensor("out_shard", [S, O], f32, kind="ExternalOutput")

    with tile.TileContext(nc) as tc:
        with tc.tile_pool(name="dram", bufs=1, space="DRAM") as dramp:
            msgbuf = dramp.tile([msg_rows, P], f16, name="msgbuf")
            h16s = dramp.tile([S, P], f16, name="h16s")
            h16f = dramp.tile([N, P], f16, addr_space="Shared", name="h16f")

            with tc.tile_pool(name="const", bufs=1) as cp:
                identf = cp.tile([P, P], f16, name="identf")
                identf32 = cp.tile([P, P], f32, name="identf32")
                make_identity(nc, identf32[:])
                nc.vector.tensor_copy(identf[:], identf32[:])
                iota_i = cp.tile([P, P], i32, name="iota_i")
                nc.gpsimd.iota(iota_i[:], pattern=[[1, P]], base=0,
                               channel_multiplier=0)
                iota_f = cp.tile([P, P], f32, name="iota_f")
                nc.vector.tensor_copy(iota_f[:], iota_i[:])
                iota_h = cp.tile([P, P], f16, name="iota_h")
                nc.vector.tensor_copy(iota_h[:], iota_i[:])
                c38i = cp.tile([P, RO], i32, name="c38i")
                nc.gpsimd.iota(c38i[:], pattern=[[1, R], [0, O]], base=0,
                               channel_multiplier=0)
                c38f = cp.tile([P, RO], f32, name="c38f")
                nc.vector.tensor_copy(c38f[:], c38i[:])
                ftTs = cp.tile([P, NT * P], f16, name="ftTs")
                nc.sync.dma_start(out=ftTs[:], in_=featT16[:])
                w1s = cp.tile([D, R * H], f16, name="w1s")
                nc.sync.dma_start(out=w1s[:], in_=w1f16[:])
                w2s = cp.tile([H, RO], f16, name="w2s")
                nc.sync.dma_start(out=w2s[:], in_=w2f16[:])
                l1s = cp.tile([D, H], f16, name="l1s")
                nc.sync.dma_start(out=l1s[:], in_=loop1d[:])
                l2s = cp.tile([H, O], f16, name="l2s")
                nc.sync.dma_start(out=l2s[:], in_=loop2d[:])
                b1s = cp.tile([P, H], f32, name="b1s")
                nc.sync.dma_start(out=b1s[:], in_=b1d[:])
                b2s = cp.tile([P, O], f32, name="b2s")
                nc.sync.dma_start(out=b2s[:], in_=b2d[:])
                g1s = cp.tile([P, T1tot * P // 16], i16, name="g1s")
                nc.sync.dma_start(out=g1s[:], in_=g1d[:])
                s2gs = cp.tile([P, T2tot * P // 16], i16, name="s2gs")
                nc.sync.dma_start(out=s2gs[:], in_=s2gd[:])
                g3s = cp.tile([P, T3tot * P // 16], i16, name="g3s")
                nc.sync.dma_start(out=g3s[:], in_=g3d[:])
                d2s = cp.tile([P, T2tot], f16, name="d2s")
                nc.sync.dma_start(out=d2s[:], in_=d2d[:])
                d3s = cp.tile([P, T3tot], f16, name="d3s")
                nc.sync.dma_start(out=d3s[:], in_=d3d[:])
                e3s = cp.tile([P, T3tot], f32, name="e3s")
                nc.sync.dma_start(out=e3s[:], in_=e3d[:])
                hT_sb = cp.tile([H, NT * P], f16, name="hT_sb")
                agg2 = cp.tile([P, NT * O], f32, name="agg2")

                msgz = cp.tile([P, P], f16, name="msgz")
                nc.vector.memset(msgz[:], 0.0)
                nc.sync.dma_start(
                    out=msgbuf[:].rearrange("(g p) e -> p g e", p=P),
                    in_=_bc_mid(msgz[:], T1tot))

                # ---------------- L1 messages ------------------------------
                with tc.tile_pool(name="p1sb", bufs=3) as sb, \
                     tc.tile_pool(name="p1pa", bufs=2, space="PSUM") as psa, \
                     tc.tile_pool(name="p1pb", bufs=2, space="PSUM") as psb:
                    for ci, (t0, nt, hsel) in enumerate(l1_chunks):
                        gat = sb.tile([P, nt * P], f16, tag="gat", name="gat")
                        src_ap = feat16[HALF:N] if (hsel and N > HALF) else feat16[:]
                        nc.gpsimd.dma_gather(
                            gat[:].rearrange("p (t e) -> p t e", e=P),
                            src_ap, g1s[:, t0 * 8:(t0 + nt) * 8],
                            nt * P, nt * P, P, elem_step=P,
                            queue_num=0)
                        stage = sb.tile([P, nt * H], f16, tag="stage",
                                        name="stage")
                        for b0 in range(0, nt, MB):
                            nb = min(MB, nt - b0)
                            gtp = psa.tile([P, MB * P], f16, tag="gtp",
                                           name="gtp")
                            gts = sb.tile([P, MB * P], f16, tag="gts",
                                          name="gts")
                            msp = psb.tile([P, MB * H], f32, tag="msp",
                                           name="msp")
                            for j in range(nb):
                                nc.tensor.transpose(
                                    out=gtp[:, j * P:(j + 1) * P],
                                    in_=gat[:, (b0 + j) * P:(b0 + j + 1) * P],
                                    identity=identf[:])
                            if b0 % (2 * MB) == 0:
                                nc.scalar.copy(out=gts[:, :nb * P],
                                               in_=gtp[:, :nb * P])
                            else:
                                nc.vector.tensor_copy(gts[:, :nb * P],
                                                      gtp[:, :nb * P])
                            for j in range(nb):
                                r = tile_rel[t0 + b0 + j]
                                nc.tensor.matmul(
                                    out=msp[:, j * H:(j + 1) * H],
                                    lhsT=gts[:, j * P:(j + 1) * P],
                                    rhs=w1s[:, r * H:(r + 1) * H],
                                    start=True, stop=True)
                            nc.scalar.copy(
                                out=stage[:, b0 * H:(b0 + nb) * H],
                                in_=msp[:, :nb * H])
                        nc.sync.dma_start(
                            out=_shrink_last(
                                msgbuf[t0 * P:(t0 + nt) * P, :].rearrange(
                                    "(g p) e -> p g e", p=P), H),
                            in_=stage[:].rearrange("p (g h) -> p g h", h=H))

                # ---------------- L1 aggregation ---------------------------
                if PHASES >= 2:
                 with tc.tile_pool(name="p2sb", bufs=3) as sb2, \
                     tc.tile_pool(name="p2pa", bufs=2, space="PSUM") as ps2a, \
                     tc.tile_pool(name="p2pb", bufs=2, space="PSUM") as ps2b:
                    cur = None
                    for ci, (t0, nt, part) in enumerate(p2_chunks):
                        mch = sb2.tile([P, nt * P], f16, tag="mch",
                                       name="mch")
                        mrows = T1p[part] * P
                        nc.gpsimd.dma_gather(
                            mch[:].rearrange("p (t e) -> p t e", e=P),
                            msgbuf[moff[part]:moff[part] + mrows, :],
                            s2gs[:, t0 * 8:(t0 + nt) * 8],
                            nt * P, nt * P, P, elem_step=P,
                            queue_num=0)
                        selb = sb2.tile([P, nt * P], f16, tag="selb",
                                        name="selb")
                        with nc.allow_low_precision(
                                reason="0/1 one-hot mask in fp16 is exact"):
                            nc.vector.tensor_tensor(
                                out=selb[:].rearrange("p (g j) -> p g j",
                                                      g=nt),
                                in0=_bc_inner(d2s[:, t0:t0 + nt], P),
                                in1=_bc_mid(iota_h[:], nt),
                                op=AO.is_equal)
                        for j in range(nt):
                            gt, k, klast, _ = p2_tiles[t0 + j]
                            if k == 0:
                                cur = ps2a.tile([P, H], f32, tag="agp",
                                               name="agp")
                            nc.tensor.matmul(
                                out=cur[:],
                                lhsT=selb[:, j * P:(j + 1) * P],
                                rhs=mch[:, j * P:j * P + H],
                                start=(k == 0), stop=False)
                            if klast:
                                nc.tensor.matmul(
                                    out=cur[:],
                                    lhsT=ftTs[:, gt * P:(gt + 1) * P],
                                    rhs=l1s[:], start=False, stop=True)
                                hb = sb2.tile([P, H], f32, tag="hb",
                                              name="hb")
                                nc.vector.tensor_tensor(
                                    out=hb[:], in0=cur[:], in1=b1s[:],
                                    op=AO.add)
                                nc.vector.tensor_scalar_max(
                                    out=hb[:], in0=hb[:], scalar1=0.0)
                                h16c = sb2.tile([P, P], f16, tag="h16c",
                                                name="h16c")
                                nc.vector.memset(h16c[:, H:P], 0.0)
                                nc.vector.tensor_copy(h16c[:, 0:H], hb[:])
                                rows = min(P, S - gt * P)
                                nc.sync.dma_start(
                                    out=h16s[gt * P:gt * P + rows, :],
                                    in_=h16c[:rows, :])
                                hTp = ps2b.tile([H, P], f32, tag="hTp",
                                               name="hTp")
                                nc.tensor.transpose(
                                    out=hTp[:], in_=hb[:],
                                    identity=identf32[:])
                                nc.scalar.copy(
                                    out=hT_sb[:, gt * P:(gt + 1) * P],
                                    in_=hTp[:])
                    if NO_COLLECTIVE:
                        nc.sync.dma_start(out=h16f[0:S, :], in_=h16s[:])
                    else:
                        nc.gpsimd.collective_compute(
                            "AllGather", AO.bypass,
                            replica_groups=[list(range(C))],
                            ins=[h16s[:].opt()], outs=[h16f[:].opt()])

                # ---------------- L2 ---------------------------------------
                if PHASES >= 3:
                 with tc.tile_pool(name="p3sb", bufs=3) as sb3, \
                     tc.tile_pool(name="p3pa", bufs=2, space="PSUM") as ps3a, \
                     tc.tile_pool(name="p3pb", bufs=2, space="PSUM") as ps3b, \
                     tc.tile_pool(name="p3pc", bufs=2, space="PSUM") as ps3c:
                    cur = None
                    for ci, (t0, nt, hsel) in enumerate(l3_chunks):
                        hg = sb3.tile([P, nt * P], f16, tag="hg", name="hg")
                        src_ap = h16f[HALF:N] if (hsel and N > HALF) else h16f[:]
                        nc.gpsimd.dma_gather(
                            hg[:].rearrange("p (t e) -> p t e", e=P),
                            src_ap, g3s[:, t0 * 8:(t0 + nt) * 8],
                            nt * P, nt * P, P, elem_step=P,
                            queue_num=0)
                        m2b = sb3.tile([P, nt * O], f16, tag="m2b",
                                       name="m2b")
                        for b0 in range(0, nt, MB):
                            nb = min(MB, nt - b0)
                            hgtp = ps3a.tile([H, MB * P], f16, tag="hgtp",
                                            name="hgtp")
                            hgt = sb3.tile([H, MB * P], f16, tag="hgt",
                                           name="hgt")
                            m38 = ps3b.tile([P, MB * RO], f32, tag="m38",
                                           name="m38")
                            for j in range(nb):
                                nc.tensor.transpose(
                                    out=hgtp[:, j * P:(j + 1) * P],
                                    in_=hg[:, (b0 + j) * P:(b0 + j) * P + H],
                                    identity=identf[:])
                            if b0 % (2 * MB) == 0:
                                nc.scalar.copy(out=hgt[:, :nb * P],
                                               in_=hgtp[:, :nb * P])
                            else:
                                nc.vector.tensor_copy(hgt[:, :nb * P],
                                                      hgtp[:, :nb * P])
                            for j in range(nb):
                                nc.tensor.matmul(
                                    out=m38[:, j * RO:(j + 1) * RO],
                                    lhsT=hgt[:, j * P:(j + 1) * P],
                                    rhs=w2s[:], start=True, stop=True)
                            mskb = sb3.tile([P, MB * RO], f32, tag="mskb",
                                            name="mskb")
                            nc.vector.tensor_tensor(
                                out=mskb[:, :nb * RO].rearrange(
                                    "p (g c) -> p g c", g=nb),
                                in0=_bc_inner(e3s[:, t0 + b0:t0 + b0 + nb],
                                              RO),
                                in1=_bc_mid(c38f[:], nb),
                                op=AO.is_equal)
                            nc.vector.tensor_tensor(
                                out=mskb[:, :nb * RO], in0=mskb[:, :nb * RO],
                                in1=m38[:, :nb * RO], op=AO.mult)
                            with nc.allow_low_precision(
                                    reason="mask picks exactly one relation;"
                                    " fp16 reduce adds one nonzero term"):
                                nc.vector.tensor_reduce(
                                    out=m2b[:, b0 * O:(b0 + nb) * O],
                                    in_=mskb[:, :nb * RO].rearrange(
                                        "p (g r o) -> p g o r", r=R, o=O),
                                    axis=mybir.AxisListType.X, op=AO.add)
                        sel2 = sb3.tile([P, nt * P], f16, tag="sel2",
                                        name="sel2")
                        with nc.allow_low_precision(
                                reason="0/1 one-hot mask in fp16 is exact"):
                            nc.vector.tensor_tensor(
                                out=sel2[:].rearrange("p (g j) -> p g j",
                                                      g=nt),
                                in0=_bc_inner(d3s[:, t0:t0 + nt], P),
                                in1=_bc_mid(iota_h[:], nt),
                                op=AO.is_equal)
                        for j in range(nt):
                            gt, k, klast, hh = l3_tiles[t0 + j]
                            if k == 0:
                                cur = ps3c.tile([P, O], f32, tag="otp",
                                               name="otp")
                            nc.tensor.matmul(
                                out=cur[:],
                                lhsT=sel2[:, j * P:(j + 1) * P],
                                rhs=m2b[:, j * O:(j + 1) * O],
                                start=(k == 0), stop=(klast and hh == 0))
                            if klast and hh == 0:
                                nc.scalar.copy(
                                    out=agg2[:, gt * O:(gt + 1) * O],
                                    in_=cur[:])
                            elif klast:
                                nc.tensor.matmul(
                                    out=cur[:],
                                    lhsT=hT_sb[:, gt * P:(gt + 1) * P],
                                    rhs=l2s[:], start=False, stop=True)
                                ob = sb3.tile([P, O], f32, tag="ob",
                                              name="ob")
                                nc.vector.tensor_tensor(
                                    out=ob[:], in0=cur[:],
                                    in1=agg2[:, gt * O:(gt + 1) * O],
                                    op=AO.add)
                                nc.vector.tensor_tensor(
                                    out=ob[:], in0=ob[:], in1=b2s[:],
                                    op=AO.add)
                                rows = min(P, S - gt * P)
                                nc.sync.dma_start(
                                    out=outs[gt * P:gt * P + rows, :],
                                    in_=ob[:rows, :])

    nc.compile()
    return nc


# ---------------------------------------------------------------- entry

def _run(in_maps, plan, trace=False):
    from concourse.bass_utils import run_bass_kernel_spmd

    key = (PHASES, NO_COLLECTIVE, plan["T1tot"], plan["T2tot"], plan["T3tot"], plan["tile_rel"],
           plan["p2_tiles"], plan["l3_tiles"])
    nc = _CACHE.get(key)
    if nc is None:
        nc = _build(plan)
        _CACHE[key] = nc
    res = run_bass_kernel_spmd(nc, in_maps, list(range(C)), trace=trace)
    out = np.concatenate([res.results[c]["out_shard"] for c in range(C)],
                         axis=0)
    return out, res


def kernel(**inputs):
    plan, in_maps = _preprocess(**inputs)
    out, _ = _run(in_maps, plan)
    return out
